# revision 12
# baseline (speedup 1.0000x reference)
"""Trainium2 Bass kernel: full-sequence multi-head attention
(S=2048, DIM=1024, H=16, D=64) sharded across 8 NeuronCores with
tensor parallelism on heads (2 heads per core), zero device collectives.

v3 design. Roofline: ACT-exp (8.4M elements/core, 64 x ~1.04us) bounds
the kernel; everything else hides under it.

Per-core device program (bf16 matmuls, f32 PSUM accumulation):
  phase 1 (ACT otherwise idle -> rope swaps live on ACT, as v1):
    rope q (chunks 0,1) then k chunk 0: qkvT matmuls (K=128, N=512)
      + ACT 32-block swap copies + DVE STT (cos/sin) + add
    scores/exp pipeline for head A starts right after k chunk 0;
    rope k chunk 1 and the v projection (v1-style N=128 MMs into
    vAB [vA(64)|1|vB(64)|1] blocks) are emitted inside the early kt
    steps to fill PE while ACT streams exps.
  phase 2 (ACT-bound kt pipeline, head A then head B):
    per (head, kt): 4 scores MMs (K=64, N=512) -> 2x [128,1024] f32
      psum (bufs=2); 2 EXP evacs -> pt [128,2048] bf16; 4 av MMs
      (K=128, M=65, N=512) accumulating into 4 [65,512] psum banks
      (half of v1's av PE cycles: v1 used K=64 pairs).
    norm per head: reciprocal_approx_fast on the [1,2048] den row
      (bf16), K=1 matmul broadcast (ones[1,64] x recip row) into
      [64,512] psum tiles, DVE multiply into outN. No shuffles, no
      DMA chains.
  tail: proj (K=128, 16 u-tiles x 2 MMs), DVE psum evac casts,
    per-u output DMA on 2 queues.
PSUM: "sc" pool 2x[128,1024] f32 (4 banks) + "wk" pool 4x[128,512]
f32 (4 banks) = 8 banks.
Host: y = sum_c y_c + b_proj in float64.
"""

import sys

if "/opt/trn_rl_repo" not in sys.path:
    sys.path.insert(0, "/opt/trn_rl_repo")

import numpy as np
import ml_dtypes

from concourse import bass, bacc, tile, bass_utils

mybir = bass.mybir
F32 = mybir.dt.float32
BF16 = mybir.dt.bfloat16
EXP = mybir.ActivationFunctionType.Exp
IDT = mybir.ActivationFunctionType.Identity
ADD = mybir.AluOpType.add
MULT = mybir.AluOpType.mult

S, DIM, H, D = 2048, 1024, 16, 64
N_CORES = 8
HPC = 2          # heads per core
DL = HPC * D     # local head dims = 128
NKT = S // 128   # 16 k tiles
NDT = DIM // 128  # 8 contraction tiles for qkv
VB = 130         # vAB block width: [vA(64) | 1 | vB(64) | 1]


DEBUG_DUMP = False


def build():
    nc = bacc.Bacc("TRN2", target_bir_lowering=False, debug=False,
                   num_devices=N_CORES)

    xT_e = nc.dram_tensor("xT", [DIM, S], BF16, kind="ExternalInput").ap()
    wqT_e = nc.dram_tensor("wqT", [DIM, DL], BF16, kind="ExternalInput").ap()
    wkT_e = nc.dram_tensor("wkT", [DIM, DL], BF16, kind="ExternalInput").ap()
    wvT_e = nc.dram_tensor("wvT", [DIM, DL], BF16, kind="ExternalInput").ap()
    cosT_e = nc.dram_tensor("cosT", [DL, S], BF16, kind="ExternalInput").ap()
    sinTs_e = nc.dram_tensor("sinTs", [DL, S], BF16, kind="ExternalInput").ap()
    wpT_e = nc.dram_tensor("wpT", [DL, DIM], BF16, kind="ExternalInput").ap()
    bq_e = nc.dram_tensor("bq", [DL, 2], F32, kind="ExternalInput").ap()
    bk_e = nc.dram_tensor("bk", [DL, 2], F32, kind="ExternalInput").ap()
    bvb_e = nc.dram_tensor("bvb", [DL, DL], F32, kind="ExternalInput").ap()
    out_e = nc.dram_tensor("out", [S, DIM], BF16, kind="ExternalOutput").ap()

    with tile.TileContext(nc) as tc:
        with tc.tile_pool(name="persist", bufs=1) as pp, \
             tc.tile_pool(name="sc", bufs=2, space="PSUM") as scp, \
             tc.tile_pool(name="wk", bufs=4, space="PSUM") as wkp, \
             tc.tile_pool(name="rope_t", bufs=2) as rtp, \
             tc.tile_pool(name="pt", bufs=7) as ptp, \
             tc.tile_pool(name="rb", bufs=2) as rbp, \
             tc.tile_pool(name="ysb", bufs=4) as ysbp:
            q_rot = pp.tile([128, S], BF16, tag="q_rot", name="q_rot")
            k_rot = pp.tile([128, S], BF16, tag="k_rot", name="k_rot")
            vAB = pp.tile([128, NKT * VB], BF16, tag="vAB", name="vAB")
            outA = pp.tile([65, S], F32, tag="outA", name="outA")
            outB = pp.tile([65, S], F32, tag="outB", name="outB")
            outN = pp.tile([128, S], BF16, tag="outN", name="outN")
            wpT = pp.tile([DL, DIM], BF16, tag="wpT", name="wpT")
            bq = pp.tile([DL, 2], F32, tag="bq", name="bq")
            bk = pp.tile([DL, 2], F32, tag="bk", name="bk")
            bvb = pp.tile([DL, DL], F32, tag="bvb", name="bvb")
            p1_cm = tc.tile_pool(name="p1in", bufs=1)
            p1 = p1_cm.__enter__()
            x_sb = [p1.tile([128, S], BF16, tag=f"x{i}", name=f"x{i}")
                    for i in range(NDT)]
            wq_sb = [p1.tile([128, DL], BF16, tag=f"wq{i}", name=f"wq{i}")
                     for i in range(NDT)]
            wk_sb = [p1.tile([128, DL], BF16, tag=f"wk{i}", name=f"wk{i}")
                     for i in range(NDT)]
            wv_sb = [p1.tile([128, DL], BF16, tag=f"wv{i}", name=f"wv{i}")
                     for i in range(NDT)]
            cosT = p1.tile([DL, S], BF16, tag="cosT", name="cosT")
            sinTs = p1.tile([DL, S], BF16, tag="sinTs", name="sinTs")

            # input DMAs: x+wq first (rope-q is first consumer), then wk,
            # tables, wv, rest. Two queues: sync + gpsimd.
            qs = [nc.sync, nc.gpsimd]
            for i in range(NDT):
                r0 = i * 128
                qs[i % 2].dma_start(x_sb[i][:], xT_e[r0:r0 + 128, :])
                qs[(i + 1) % 2].dma_start(wq_sb[i][:], wqT_e[r0:r0 + 128, :])
            for i in range(NDT):
                r0 = i * 128
                qs[i % 2].dma_start(wk_sb[i][:], wkT_e[r0:r0 + 128, :])
            nc.gpsimd.dma_start(bq[:], bq_e[:])
            nc.gpsimd.dma_start(bk[:], bk_e[:])
            nc.sync.dma_start(cosT[:], cosT_e[:])
            nc.sync.dma_start(sinTs[:], sinTs_e[:])
            for i in range(NDT):
                r0 = i * 128
                qs[i % 2].dma_start(wv_sb[i][:], wvT_e[r0:r0 + 128, :])
            nc.gpsimd.dma_start(bvb[:], bvb_e[:])
            nc.gpsimd.dma_start(wpT[:], wpT_e[:])

            # ones columns of vAB (cols t*130+64 and t*130+129)
            v3 = vAB[:].rearrange("p (t c) -> p t c", c=65)  # [128, 32, 65]
            nc.vector.memset(v3[:, :, 64:65], 1.0)

            # ---------------- rope (v1-style, ACT swaps) ----------------
            def rope_pass(w_sb, bias, dest, cp):
                cs = cp * 1024
                ps = scp.tile([128, 1024], F32, tag="sc", name="sc")
                for i in range(NDT):
                    for h in range(2):
                        nc.tensor.matmul(
                            ps[:, h * 512:(h + 1) * 512], w_sb[i][:],
                            x_sb[i][:, cs + h * 512:cs + (h + 1) * 512],
                            start=(i == 0), stop=(i == NDT - 1))
                qsw = rtp.tile([128, 1024], F32, tag="qsw", name="qsw")
                t1 = rtp.tile([128, 1024], F32, tag="t1", name="t1")
                nc.scalar.activation(qsw[0:32, :], ps[32:64, :], IDT)
                nc.scalar.activation(qsw[32:64, :], ps[0:32, :], IDT)
                nc.scalar.activation(qsw[64:96, :], ps[96:128, :], IDT)
                nc.scalar.activation(qsw[96:128, :], ps[64:96, :], IDT)
                nc.vector.scalar_tensor_tensor(
                    t1[:], ps[:], bias[:, 0:1], cosT[:, cs:cs + 1024],
                    op0=ADD, op1=MULT)
                nc.vector.scalar_tensor_tensor(
                    qsw[:], qsw[:], bias[:, 1:2],
                    sinTs[:, cs:cs + 1024], op0=ADD, op1=MULT)
                nc.vector.tensor_add(dest[:, cs:cs + 1024], t1[:], qsw[:])

            def v_tiles(ts_range):
                # v1-style: v in [k, d] layout via N=128 MMs + bias add
                # into the interleaved vAB blocks.
                for t in ts_range:
                    ps = wkp.tile([128, 512], F32, tag="wk", name="wkv")
                    for i in range(NDT):
                        nc.tensor.matmul(
                            ps[:, 0:128],
                            x_sb[i][:, t * 128:(t + 1) * 128],
                            wv_sb[i][:],
                            start=(i == 0), stop=(i == NDT - 1))
                    blk = vAB[:, t * VB:(t + 1) * VB].rearrange(
                        "p (b c) -> p b c", c=65)
                    nc.vector.tensor_add(
                        blk[:, :, 0:64],
                        ps[:, 0:128].rearrange("p (b c) -> p b c", c=64),
                        bvb[:].rearrange("p (b c) -> p b c", c=64))

            # ---------------- phase 2 helpers ----------------
            av_ps = {}

            def emit_scores_exp(h, kt):
                hp = h * 64
                pt = ptp.tile([128, S], BF16, tag="pt", name="pt")
                for half in range(2):
                    ps = scp.tile([128, 1024], F32, tag="sc", name="sc")
                    for j in range(2):
                        q0 = half * 1024 + j * 512
                        nc.tensor.matmul(
                            ps[:, j * 512:(j + 1) * 512],
                            k_rot[hp:hp + 64, kt * 128:(kt + 1) * 128],
                            q_rot[hp:hp + 64, q0:q0 + 512],
                            start=True, stop=True)
                    nc.scalar.activation(
                        pt[:, half * 1024:(half + 1) * 1024], ps[:], EXP)
                return pt

            def emit_av(h, kt, pt):
                vc0 = kt * VB + h * 65
                for cc in range(4):
                    if (h, cc) not in av_ps:
                        av_ps[(h, cc)] = wkp.tile([128, 512], F32, tag="wk",
                                                  name=f"av{h}{cc}")
                    nc.tensor.matmul(
                        av_ps[(h, cc)][0:65, :],
                        vAB[:, vc0:vc0 + 65],
                        pt[:, cc * 512:(cc + 1) * 512],
                        start=(kt == 0), stop=(kt == NKT - 1))

            def evac_av(h, o_sb):
                for cc in range(4):
                    nc.vector.tensor_copy(
                        o_sb[:, cc * 512:(cc + 1) * 512],
                        av_ps.pop((h, cc))[0:65, :])

            def emit_norm(h, o_sb):
                # GPSIMD partition-broadcast of the raw den row to 64
                # partitions (idle engine, one op), DVE recip on the
                # [64, S] tile, DVE multiply into outN.
                base = h * 64
                rr = rbp.tile([1, S], F32, tag="rr", name="rr")
                bc = rbp.tile([64, S], F32, tag="bc", name="bc")
                nc.vector.tensor_copy(rr[:], o_sb[64:65, :])
                nc.gpsimd.partition_broadcast(bc[:], rr[:])
                if DEBUG_DUMP:
                    e = nc.dram_tensor(f"d_bc{h}", [64, S], F32,
                                       kind="ExternalOutput").ap()
                    nc.sync.dma_start(e[:], bc[:])
                nc.vector.reciprocal_approx_fast(bc[:], bc[:])
                if DEBUG_DUMP:
                    e = nc.dram_tensor(f"d_br{h}", [64, S], F32,
                                       kind="ExternalOutput").ap()
                    nc.sync.dma_start(e[:], bc[:])
                nc.vector.tensor_mul(
                    outN[base:base + 64, :], o_sb[0:64, :], bc[:])

            def emit_proj():
                for u in range(16):
                    ss = u * 128
                    pja = wkp.tile([128, 512], F32, tag="wk", name=f"pj{u}a")
                    pjb = wkp.tile([128, 512], F32, tag="wk", name=f"pj{u}b")
                    nc.tensor.matmul(pja[:], outN[:, ss:ss + 128],
                                     wpT[:, 0:512], start=True, stop=True)
                    nc.tensor.matmul(pjb[:], outN[:, ss:ss + 128],
                                     wpT[:, 512:1024], start=True, stop=True)
                    ysb = ysbp.tile([128, 1024], BF16, tag="ysb", name="ysb")
                    nc.vector.tensor_copy(ysb[:, 0:512], pja[:])
                    nc.vector.tensor_copy(ysb[:, 512:1024], pjb[:])
                    qs[u % 2].dma_start(out_e[ss:ss + 128, :], ysb[:])

            # ---------------- schedule ----------------
            rope_pass(wq_sb, bq, q_rot, 0)
            rope_pass(wq_sb, bq, q_rot, 1)
            rope_pass(wk_sb, bk, k_rot, 0)

            # head A: exp stream starts on kt 0..3 (needs only k chunk 0);
            # rope k chunk 1 and the v tiles fill PE behind the stream.
            pts = {}
            for kt in range(2):
                pts[kt] = emit_scores_exp(0, kt)
            rope_pass(wk_sb, bk, k_rot, 1)
            for kt in range(2, 4):
                pts[kt] = emit_scores_exp(0, kt)
            v_tiles(range(0, 8))
            for kt in range(4, 6):
                pts[kt] = emit_scores_exp(0, kt)
            v_tiles(range(8, NKT))
            emit_av(0, 0, pts.pop(0))
            for kt in range(6, NKT):
                pts[kt] = emit_scores_exp(0, kt)
                emit_av(0, kt - 5, pts.pop(kt - 5))
            for kt in range(NKT - 5, NKT):
                emit_av(0, kt, pts.pop(kt))
            evac_av(0, outA)
            emit_norm(0, outA)

            # head B kt loop
            pts[0] = emit_scores_exp(1, 0)
            pts[1] = emit_scores_exp(1, 1)
            emit_av(1, 0, pts.pop(0))
            for kt in range(2, NKT):
                pts[kt] = emit_scores_exp(1, kt)
                emit_av(1, kt - 1, pts.pop(kt - 1))
            emit_av(1, NKT - 1, pts.pop(NKT - 1))
            evac_av(1, outB)
            emit_norm(1, outB)

            emit_proj()

            if DEBUG_DUMP:
                dbg = {
                    "d_vAB": (vAB, [128, NKT * VB], BF16),
                    "d_krot": (k_rot, [128, S], BF16),
                    "d_qrot": (q_rot, [128, S], BF16),
                    "d_outA": (outA, [65, S], F32),
                    "d_outB": (outB, [65, S], F32),
                    "d_outN": (outN, [128, S], BF16),
                }
                for name, (t, shape, dt) in dbg.items():
                    e = nc.dram_tensor(name, shape, dt,
                                       kind="ExternalOutput").ap()
                    nc.sync.dma_start(e[:], t[:])

            p1_cm.__exit__(None, None, None)

    nc.compile()
    return nc


def make_in_maps(x, sin, cos, W_qkv, b_qkv):
    x = np.asarray(x, np.float32)
    sin = np.asarray(sin, np.float32)
    cos = np.asarray(cos, np.float32)
    W_qkv = np.asarray(W_qkv, np.float32)
    b_qkv = np.asarray(b_qkv, np.float32)

    xT = np.ascontiguousarray(x.T).astype(ml_dtypes.bfloat16)
    # sin/cos halves are duplicated (ang = concat([ang, ang])); rows are
    # [h0 d0:32, h0 d32:64, h1 d0:32, h1 d32:64] -> 4x tile of the
    # first-half columns works for cos. The rotate-half sign pattern is
    # [-s, +s, -s, +s] per 32-row block.
    cosT = np.ascontiguousarray(
        np.tile(cos[:, :32].T, (4, 1))).astype(ml_dtypes.bfloat16)
    sin32 = sin[:, :32].T
    sinTs = np.ascontiguousarray(
        np.concatenate([-sin32, sin32, -sin32, sin32], 0)).astype(
            ml_dtypes.bfloat16)

    scale = 1.0 / np.sqrt(np.float32(D))
    Wq = W_qkv[0:DIM] * scale
    Wk = W_qkv[DIM:2 * DIM]
    Wv = W_qkv[2 * DIM:3 * DIM]
    bq_full = b_qkv[0:DIM] * scale
    bk_full = b_qkv[DIM:2 * DIM]
    bv_full = b_qkv[2 * DIM:3 * DIM]

    in_maps = []
    for core in range(N_CORES):
        h0, h1 = 2 * core, 2 * core + 1

        def head_rows(W):
            return np.concatenate([W[h0 * D:(h0 + 1) * D],
                                   W[h1 * D:(h1 + 1) * D]], 0)

        def swap32(b):
            return np.concatenate([b[32:64], b[0:32], b[96:128], b[64:96]], 0)

        wq_c = head_rows(Wq)
        wk_c = head_rows(Wk)
        wv_c = head_rows(Wv)
        bq_c = head_rows(bq_full[:, None])[:, 0]
        bk_c = head_rows(bk_full[:, None])[:, 0]
        bq2 = np.stack([bq_c, swap32(bq_c)], 1)
        bk2 = np.stack([bk_c, swap32(bk_c)], 1)
        bv_row = head_rows(bv_full[:, None])[:, 0]
        bvb_c = np.broadcast_to(bv_row[None, :], (DL, DL))
        in_maps.append({
            "xT": xT,
            "wqT": np.ascontiguousarray(wq_c.T).astype(ml_dtypes.bfloat16),
            "wkT": np.ascontiguousarray(wk_c.T).astype(ml_dtypes.bfloat16),
            "wvT": np.ascontiguousarray(wv_c.T).astype(ml_dtypes.bfloat16),
            "cosT": cosT,
            "sinTs": sinTs,
            "bq": np.ascontiguousarray(bq2),
            "bk": np.ascontiguousarray(bk2),
            "bvb": np.ascontiguousarray(bvb_c),
        })
    return in_maps


def add_wp(in_maps, W_proj):
    W_proj = np.asarray(W_proj, np.float32)
    for core in range(N_CORES):
        cols = slice(core * DL, (core + 1) * DL)
        in_maps[core]["wpT"] = np.ascontiguousarray(
            W_proj[:, cols].T).astype(ml_dtypes.bfloat16)
    return in_maps


_NC_CACHE = {}


def kernel(x, sin, cos, W_qkv, b_qkv, W_proj, b_proj):
    if "nc" not in _NC_CACHE:
        _NC_CACHE["nc"] = build()
    nc = _NC_CACHE["nc"]
    in_maps = add_wp(make_in_maps(x, sin, cos, W_qkv, b_qkv), W_proj)
    res = bass_utils.run_bass_kernel_spmd(
        nc, in_maps, core_ids=list(range(N_CORES)))
    y = np.zeros((S, DIM), np.float64)
    for core in range(N_CORES):
        y += res.results[core]["out"].astype(np.float64)
    y += np.asarray(b_proj, np.float32)[None, :].astype(np.float64)
    return y.astype(np.float32)


# revision 15
# speedup vs baseline: 1.1088x; 1.1088x over previous
"""Trainium2 Bass kernel: full-sequence multi-head attention
(S=2048, DIM=1024, H=16, D=64) sharded across 8 NeuronCores with
tensor parallelism on heads (2 heads per core), zero device collectives.

v3 design. Roofline: ACT-exp (8.4M elements/core, 64 x ~1.04us) bounds
the kernel; everything else hides under it.

Per-core device program (bf16 matmuls, f32 PSUM accumulation):
  phase 1 (ACT otherwise idle -> rope swaps live on ACT, as v1):
    rope q (chunks 0,1) then k chunk 0: qkvT matmuls (K=128, N=512)
      + ACT 32-block swap copies + DVE STT (cos/sin) + add
    scores/exp pipeline for head A starts right after k chunk 0;
    rope k chunk 1 and the v projection (v1-style N=128 MMs into
    vAB [vA(64)|1|vB(64)|1] blocks) are emitted inside the early kt
    steps to fill PE while ACT streams exps.
  phase 2 (ACT-bound kt pipeline, head A then head B):
    per (head, kt): 4 scores MMs (K=64, N=512) -> 2x [128,1024] f32
      psum (bufs=2); 2 EXP evacs -> pt [128,2048] bf16; 4 av MMs
      (K=128, M=65, N=512) accumulating into 4 [65,512] psum banks
      (half of v1's av PE cycles: v1 used K=64 pairs).
    norm per head: reciprocal_approx_fast on the [1,2048] den row
      (bf16), K=1 matmul broadcast (ones[1,64] x recip row) into
      [64,512] psum tiles, DVE multiply into outN. No shuffles, no
      DMA chains.
  tail: proj (K=128, 16 u-tiles x 2 MMs), DVE psum evac casts,
    per-u output DMA on 2 queues.
PSUM: "sc" pool 2x[128,1024] f32 (4 banks) + "wk" pool 4x[128,512]
f32 (4 banks) = 8 banks.
Host: y = sum_c y_c + b_proj in float64.
"""

import sys

if "/opt/trn_rl_repo" not in sys.path:
    sys.path.insert(0, "/opt/trn_rl_repo")

import numpy as np
import ml_dtypes

from concourse import bass, bacc, tile, bass_utils

mybir = bass.mybir
F32 = mybir.dt.float32
BF16 = mybir.dt.bfloat16
EXP = mybir.ActivationFunctionType.Exp
IDT = mybir.ActivationFunctionType.Identity
ADD = mybir.AluOpType.add
MULT = mybir.AluOpType.mult

S, DIM, H, D = 2048, 1024, 16, 64
N_CORES = 8
HPC = 2          # heads per core
DL = HPC * D     # local head dims = 128
NKT = S // 128   # 16 k tiles
NDT = DIM // 128  # 8 contraction tiles for qkv
VB = 130         # vAB block width: [vA(64) | 1 | vB(64) | 1]


DEBUG_DUMP = False


def build():
    nc = bacc.Bacc("TRN2", target_bir_lowering=False, debug=False,
                   num_devices=N_CORES)

    xT_e = nc.dram_tensor("xT", [DIM, S], BF16, kind="ExternalInput").ap()
    wqT_e = nc.dram_tensor("wqT", [DIM, DL], BF16, kind="ExternalInput").ap()
    wkT_e = nc.dram_tensor("wkT", [DIM, DL], BF16, kind="ExternalInput").ap()
    wvT_e = nc.dram_tensor("wvT", [DIM, DL], BF16, kind="ExternalInput").ap()
    cosT_e = nc.dram_tensor("cosT", [DL, S], BF16, kind="ExternalInput").ap()
    sinTs_e = nc.dram_tensor("sinTs", [DL, S], BF16, kind="ExternalInput").ap()
    wpT_e = nc.dram_tensor("wpT", [DL, DIM], BF16, kind="ExternalInput").ap()
    bq_e = nc.dram_tensor("bq", [DL, 2], F32, kind="ExternalInput").ap()
    bk_e = nc.dram_tensor("bk", [DL, 2], F32, kind="ExternalInput").ap()
    bvb_e = nc.dram_tensor("bvb", [DL, DL], F32, kind="ExternalInput").ap()
    out_e = nc.dram_tensor("out", [S, DIM], BF16, kind="ExternalOutput").ap()

    with tile.TileContext(nc) as tc:
        with tc.tile_pool(name="persist", bufs=1) as pp, \
             tc.tile_pool(name="sc", bufs=2, space="PSUM") as scp, \
             tc.tile_pool(name="wk", bufs=4, space="PSUM") as wkp, \
             tc.tile_pool(name="rope_t", bufs=2) as rtp, \
             tc.tile_pool(name="pt", bufs=10) as ptp, \
             tc.tile_pool(name="rb", bufs=2) as rbp, \
             tc.tile_pool(name="ysb", bufs=4) as ysbp:
            q_rot = pp.tile([128, S], BF16, tag="q_rot", name="q_rot")
            k_rot = pp.tile([128, S], BF16, tag="k_rot", name="k_rot")
            vAB = pp.tile([128, NKT * VB], BF16, tag="vAB", name="vAB")
            outA = pp.tile([65, S], F32, tag="outA", name="outA")
            outB = pp.tile([65, S], F32, tag="outB", name="outB")
            outN = pp.tile([128, S], BF16, tag="outN", name="outN")
            wpT = pp.tile([DL, DIM], BF16, tag="wpT", name="wpT")
            bq = pp.tile([DL, 2], F32, tag="bq", name="bq")
            bk = pp.tile([DL, 2], F32, tag="bk", name="bk")
            bvb = pp.tile([DL, DL], F32, tag="bvb", name="bvb")
            p1_cm = tc.tile_pool(name="p1in", bufs=1)
            p1 = p1_cm.__enter__()
            x_sb = [p1.tile([128, S], BF16, tag=f"x{i}", name=f"x{i}")
                    for i in range(NDT)]
            wq_sb = [p1.tile([128, DL], BF16, tag=f"wq{i}", name=f"wq{i}")
                     for i in range(NDT)]
            wk_sb = [p1.tile([128, DL], BF16, tag=f"wk{i}", name=f"wk{i}")
                     for i in range(NDT)]
            wv_sb = [p1.tile([128, DL], BF16, tag=f"wv{i}", name=f"wv{i}")
                     for i in range(NDT)]
            cosT = p1.tile([DL, S], BF16, tag="cosT", name="cosT")
            sinTs = p1.tile([DL, S], BF16, tag="sinTs", name="sinTs")

            # input DMAs: small wq/bias/table tensors first so the first
            # rope MM can issue ~2us in; x+wk interleaved behind them.
            qs = [nc.sync, nc.gpsimd]
            for i in range(NDT):
                qs[i % 2].dma_start(wq_sb[i][:],
                                    wqT_e[i * 128:(i + 1) * 128, :])
            nc.gpsimd.dma_start(bq[:], bq_e[:])
            nc.gpsimd.dma_start(bk[:], bk_e[:])
            nc.sync.dma_start(cosT[:], cosT_e[:])
            nc.sync.dma_start(sinTs[:], sinTs_e[:])
            for i in range(NDT):
                r0 = i * 128
                qs[i % 2].dma_start(x_sb[i][:], xT_e[r0:r0 + 128, :])
                qs[(i + 1) % 2].dma_start(wk_sb[i][:], wkT_e[r0:r0 + 128, :])
            for i in range(NDT):
                r0 = i * 128
                qs[i % 2].dma_start(wv_sb[i][:], wvT_e[r0:r0 + 128, :])
            nc.gpsimd.dma_start(bvb[:], bvb_e[:])
            nc.gpsimd.dma_start(wpT[:], wpT_e[:])

            # ones columns of vAB (cols t*130+64 and t*130+129)
            v3 = vAB[:].rearrange("p (t c) -> p t c", c=65)  # [128, 32, 65]
            nc.vector.memset(v3[:, :, 64:65], 1.0)

            # ---------------- rope (v1-style, ACT swaps) ----------------
            def rope_pass(w_sb, bias, dest, cp):
                cs = cp * 1024
                ps = scp.tile([128, 1024], F32, tag="sc", name="sc")
                for i in range(NDT):
                    for h in range(2):
                        nc.tensor.matmul(
                            ps[:, h * 512:(h + 1) * 512], w_sb[i][:],
                            x_sb[i][:, cs + h * 512:cs + (h + 1) * 512],
                            start=(i == 0), stop=(i == NDT - 1))
                qsw = rtp.tile([128, 1024], F32, tag="qsw", name="qsw")
                t1 = rtp.tile([128, 1024], F32, tag="t1", name="t1")
                nc.scalar.activation(qsw[0:32, :], ps[32:64, :], IDT)
                nc.scalar.activation(qsw[32:64, :], ps[0:32, :], IDT)
                nc.scalar.activation(qsw[64:96, :], ps[96:128, :], IDT)
                nc.scalar.activation(qsw[96:128, :], ps[64:96, :], IDT)
                nc.vector.scalar_tensor_tensor(
                    t1[:], ps[:], bias[:, 0:1], cosT[:, cs:cs + 1024],
                    op0=ADD, op1=MULT)
                nc.vector.scalar_tensor_tensor(
                    qsw[:], qsw[:], bias[:, 1:2],
                    sinTs[:, cs:cs + 1024], op0=ADD, op1=MULT)
                nc.vector.tensor_add(dest[:, cs:cs + 1024], t1[:], qsw[:])

            def v_tiles(ts_range):
                # v1-style: v in [k, d] layout via N=128 MMs + bias add
                # into the interleaved vAB blocks.
                for t in ts_range:
                    ps = wkp.tile([128, 512], F32, tag="wk", name="wkv")
                    for i in range(NDT):
                        nc.tensor.matmul(
                            ps[:, 0:128],
                            x_sb[i][:, t * 128:(t + 1) * 128],
                            wv_sb[i][:],
                            start=(i == 0), stop=(i == NDT - 1))
                    blk = vAB[:, t * VB:(t + 1) * VB].rearrange(
                        "p (b c) -> p b c", c=65)
                    nc.vector.tensor_add(
                        blk[:, :, 0:64],
                        ps[:, 0:128].rearrange("p (b c) -> p b c", c=64),
                        bvb[:].rearrange("p (b c) -> p b c", c=64))

            # ---------------- phase 2 helpers ----------------
            av_ps = {}

            def emit_scores_exp(h, kt):
                hp = h * 64
                pt = ptp.tile([128, S], BF16, tag="pt", name="pt")
                for half in range(2):
                    ps = scp.tile([128, 1024], F32, tag="sc", name="sc")
                    for j in range(2):
                        q0 = half * 1024 + j * 512
                        nc.tensor.matmul(
                            ps[:, j * 512:(j + 1) * 512],
                            k_rot[hp:hp + 64, kt * 128:(kt + 1) * 128],
                            q_rot[hp:hp + 64, q0:q0 + 512],
                            start=True, stop=True)
                    nc.scalar.activation(
                        pt[:, half * 1024:(half + 1) * 1024], ps[:], EXP)
                return pt

            def emit_av(h, kt, pt):
                vc0 = kt * VB + h * 65
                for cc in range(4):
                    if (h, cc) not in av_ps:
                        av_ps[(h, cc)] = wkp.tile([128, 512], F32, tag="wk",
                                                  name=f"av{h}{cc}")
                    nc.tensor.matmul(
                        av_ps[(h, cc)][0:65, :],
                        vAB[:, vc0:vc0 + 65],
                        pt[:, cc * 512:(cc + 1) * 512],
                        start=(kt == 0), stop=(kt == NKT - 1))

            def evac_av(h, o_sb):
                for cc in range(4):
                    nc.vector.tensor_copy(
                        o_sb[:, cc * 512:(cc + 1) * 512],
                        av_ps.pop((h, cc))[0:65, :])

            def emit_norm(h, o_sb):
                # GPSIMD partition-broadcast of the raw den row to 64
                # partitions (idle engine, one op), DVE recip on the
                # [64, S] tile, DVE multiply into outN.
                base = h * 64
                rr = rbp.tile([1, S], F32, tag="rr", name="rr")
                bc = rbp.tile([64, S], F32, tag="bc", name="bc")
                nc.vector.tensor_copy(rr[:], o_sb[64:65, :])
                nc.gpsimd.partition_broadcast(bc[:], rr[:])
                if DEBUG_DUMP:
                    e = nc.dram_tensor(f"d_bc{h}", [64, S], F32,
                                       kind="ExternalOutput").ap()
                    nc.sync.dma_start(e[:], bc[:])
                nc.vector.reciprocal_approx_fast(bc[:], bc[:])
                if DEBUG_DUMP:
                    e = nc.dram_tensor(f"d_br{h}", [64, S], F32,
                                       kind="ExternalOutput").ap()
                    nc.sync.dma_start(e[:], bc[:])
                nc.vector.tensor_mul(
                    outN[base:base + 64, :], o_sb[0:64, :], bc[:])

            def emit_proj():
                for u in range(16):
                    ss = u * 128
                    pja = wkp.tile([128, 512], F32, tag="wk", name=f"pj{u}a")
                    pjb = wkp.tile([128, 512], F32, tag="wk", name=f"pj{u}b")
                    nc.tensor.matmul(pja[:], outN[:, ss:ss + 128],
                                     wpT[:, 0:512], start=True, stop=True)
                    nc.tensor.matmul(pjb[:], outN[:, ss:ss + 128],
                                     wpT[:, 512:1024], start=True, stop=True)
                    ysb = ysbp.tile([128, 1024], BF16, tag="ysb", name="ysb")
                    nc.vector.tensor_copy(ysb[:, 0:512], pja[:])
                    nc.vector.tensor_copy(ysb[:, 512:1024], pjb[:])
                    qs[u % 2].dma_start(out_e[ss:ss + 128, :], ysb[:])

            # ---------------- schedule ----------------
            rope_pass(wq_sb, bq, q_rot, 0)
            rope_pass(wq_sb, bq, q_rot, 1)
            rope_pass(wk_sb, bk, k_rot, 0)

            # kt loops: scores/exp stream at ACT pace; av matmuls batched
            # per 4 kt into 16-MM contiguous bursts (HAM re-warm windows).
            # Head A additionally absorbs rope-k chunk 1 and the v tiles
            # as PE filler before the first av batch.
            pts = {}

            def av_batch(h, o_sb, kts):
                for kt in kts:
                    emit_av(h, kt, pts.pop(kt))
                if kts[-1] == NKT - 1:
                    evac_av(h, o_sb)
                    emit_norm(h, o_sb)

            for h, o_sb in ((0, outA), (1, outB)):
                for kt in range(NKT):
                    pts[kt] = emit_scores_exp(h, kt)
                    if h == 0:
                        if kt == 1:
                            rope_pass(wk_sb, bk, k_rot, 1)
                        elif kt == 2:
                            v_tiles(range(0, 8))
                        elif kt == 4:
                            v_tiles(range(8, NKT))
                    if kt == 7 or kt == 11:
                        av_batch(h, o_sb, list(range(kt - 7, kt - 3)))
                av_batch(h, o_sb, list(range(NKT - 8, NKT - 4)))
                av_batch(h, o_sb, list(range(NKT - 4, NKT)))

            emit_proj()

            if DEBUG_DUMP:
                dbg = {
                    "d_vAB": (vAB, [128, NKT * VB], BF16),
                    "d_krot": (k_rot, [128, S], BF16),
                    "d_qrot": (q_rot, [128, S], BF16),
                    "d_outA": (outA, [65, S], F32),
                    "d_outB": (outB, [65, S], F32),
                    "d_outN": (outN, [128, S], BF16),
                }
                for name, (t, shape, dt) in dbg.items():
                    e = nc.dram_tensor(name, shape, dt,
                                       kind="ExternalOutput").ap()
                    nc.sync.dma_start(e[:], t[:])

            p1_cm.__exit__(None, None, None)

    nc.compile()
    return nc


def make_in_maps(x, sin, cos, W_qkv, b_qkv):
    x = np.asarray(x, np.float32)
    sin = np.asarray(sin, np.float32)
    cos = np.asarray(cos, np.float32)
    W_qkv = np.asarray(W_qkv, np.float32)
    b_qkv = np.asarray(b_qkv, np.float32)

    xT = np.ascontiguousarray(x.T).astype(ml_dtypes.bfloat16)
    # sin/cos halves are duplicated (ang = concat([ang, ang])); rows are
    # [h0 d0:32, h0 d32:64, h1 d0:32, h1 d32:64] -> 4x tile of the
    # first-half columns works for cos. The rotate-half sign pattern is
    # [-s, +s, -s, +s] per 32-row block.
    cosT = np.ascontiguousarray(
        np.tile(cos[:, :32].T, (4, 1))).astype(ml_dtypes.bfloat16)
    sin32 = sin[:, :32].T
    sinTs = np.ascontiguousarray(
        np.concatenate([-sin32, sin32, -sin32, sin32], 0)).astype(
            ml_dtypes.bfloat16)

    scale = 1.0 / np.sqrt(np.float32(D))
    Wq = W_qkv[0:DIM] * scale
    Wk = W_qkv[DIM:2 * DIM]
    Wv = W_qkv[2 * DIM:3 * DIM]
    bq_full = b_qkv[0:DIM] * scale
    bk_full = b_qkv[DIM:2 * DIM]
    bv_full = b_qkv[2 * DIM:3 * DIM]

    in_maps = []
    for core in range(N_CORES):
        h0, h1 = 2 * core, 2 * core + 1

        def head_rows(W):
            return np.concatenate([W[h0 * D:(h0 + 1) * D],
                                   W[h1 * D:(h1 + 1) * D]], 0)

        def swap32(b):
            return np.concatenate([b[32:64], b[0:32], b[96:128], b[64:96]], 0)

        wq_c = head_rows(Wq)
        wk_c = head_rows(Wk)
        wv_c = head_rows(Wv)
        bq_c = head_rows(bq_full[:, None])[:, 0]
        bk_c = head_rows(bk_full[:, None])[:, 0]
        bq2 = np.stack([bq_c, swap32(bq_c)], 1)
        bk2 = np.stack([bk_c, swap32(bk_c)], 1)
        bv_row = head_rows(bv_full[:, None])[:, 0]
        bvb_c = np.broadcast_to(bv_row[None, :], (DL, DL))
        in_maps.append({
            "xT": xT,
            "wqT": np.ascontiguousarray(wq_c.T).astype(ml_dtypes.bfloat16),
            "wkT": np.ascontiguousarray(wk_c.T).astype(ml_dtypes.bfloat16),
            "wvT": np.ascontiguousarray(wv_c.T).astype(ml_dtypes.bfloat16),
            "cosT": cosT,
            "sinTs": sinTs,
            "bq": np.ascontiguousarray(bq2),
            "bk": np.ascontiguousarray(bk2),
            "bvb": np.ascontiguousarray(bvb_c),
        })
    return in_maps


def add_wp(in_maps, W_proj):
    W_proj = np.asarray(W_proj, np.float32)
    for core in range(N_CORES):
        cols = slice(core * DL, (core + 1) * DL)
        in_maps[core]["wpT"] = np.ascontiguousarray(
            W_proj[:, cols].T).astype(ml_dtypes.bfloat16)
    return in_maps


_NC_CACHE = {}


def kernel(x, sin, cos, W_qkv, b_qkv, W_proj, b_proj):
    if "nc" not in _NC_CACHE:
        _NC_CACHE["nc"] = build()
    nc = _NC_CACHE["nc"]
    in_maps = add_wp(make_in_maps(x, sin, cos, W_qkv, b_qkv), W_proj)
    res = bass_utils.run_bass_kernel_spmd(
        nc, in_maps, core_ids=list(range(N_CORES)))
    y = np.zeros((S, DIM), np.float64)
    for core in range(N_CORES):
        y += res.results[core]["out"].astype(np.float64)
    y += np.asarray(b_proj, np.float32)[None, :].astype(np.float64)
    return y.astype(np.float32)
